# revision 14
# baseline (speedup 1.0000x reference)
"""Trainium2 Bass kernel for nn_Decoder_72438918414350.

Single decoder step: embedding lookup + Bahdanau additive attention over
encoder outputs + LSTM cell + vocab projection.

Sharding: data-parallel over batch across 8 NeuronCores (32 rows/core).
All weights are replicated per core; matmul operands in fp16 with fp32
accumulation in PSUM; gate math and outputs in fp32. The host supplies the
encoder output both in natural layout (for the attention-weighted context
sum) and pre-transposed per bt-tile (for the E-contraction of the score
projection), so no on-chip transposes of the large tensor are needed.

Self-contained: hardcodes all shapes; only imports the installed concourse
runtime.
"""

import os
import sys
from contextlib import ExitStack

import numpy as np


def _ensure_concourse():
    try:
        import concourse.bass  # noqa: F401
    except ImportError:
        for p in ("/opt/trn_rl_repo", "/root/.axon_site/_ro/trn_rl_repo"):
            if os.path.isdir(p) and p not in sys.path:
                sys.path.insert(0, p)
        import concourse.bass  # noqa: F401


P = 128
B, T, E, D, A, V, EMB = 256, 64, 2048, 512, 512, 10000, 256
NCORES = 8
BL = B // NCORES            # 32 batch rows per core
BT = BL * T                 # 2048 (batch*time rows per core)
NBT = BT // P               # 16 bt tiles
NE = E // P                 # 16 e tiles
EI = E + EMB                # 2304
NEI = EI // P               # 18
G4 = 4 * D                  # 2048
ND = D // P                 # 4
NV512 = (V + 511) // 512    # 20 vocab chunks of <=512

_CACHE = {}
LAST_RESULTS = None


def _build():
    """Build + compile the Bass module (cached)."""
    if "nc" in _CACHE:
        return _CACHE["nc"]

    import concourse.bass as bass
    import concourse.mybir as mybir
    import concourse.tile as tile
    from concourse import bacc

    dt = mybir.dt
    AF = mybir.ActivationFunctionType
    ALU = mybir.AluOpType
    F16 = dt.float16
    F32 = dt.float32

    nc = bacc.Bacc("TRN2", target_bir_lowering=False, debug=False)

    # ---- DRAM tensors (per-core shard shapes) ----
    enc_d = nc.dram_tensor("enc", [BT, E], F16, kind="ExternalInput").ap()
    # encT[j, p, t*128+c] = enc[j*128+c, t*128+p]: per-tile transposed, partition-major
    encT_d = nc.dram_tensor("encT", [NBT, P, E], F16, kind="ExternalInput").ap()
    ht_d = nc.dram_tensor("ht", [D, BL], F16, kind="ExternalInput").ap()
    carry_d = nc.dram_tensor("carry", [BL, D], F32, kind="ExternalInput").ap()
    idx_d = nc.dram_tensor("idx", [BL, 1], dt.int32, kind="ExternalInput").ap()
    emb_d = nc.dram_tensor("emb", [V, EMB], F16, kind="ExternalInput").ap()
    w1_d = nc.dram_tensor("w1", [D, A], F16, kind="ExternalInput").ap()
    w2_d = nc.dram_tensor("w2", [E, A], F16, kind="ExternalInput").ap()
    wx_d = nc.dram_tensor("wx", [EI, G4], F16, kind="ExternalInput").ap()
    wh_d = nc.dram_tensor("wh", [D, G4], F16, kind="ExternalInput").ap()
    wp_d = nc.dram_tensor("wp", [D, V], F16, kind="ExternalInput").ap()
    vrow_d = nc.dram_tensor("vrow", [P, A], F16, kind="ExternalInput").ap()
    b12_d = nc.dram_tensor("b12", [1, A], F16, kind="ExternalInput").ap()
    bvb_d = nc.dram_tensor("bvb", [P, 1], F32, kind="ExternalInput").ap()
    bl_d = nc.dram_tensor("bl", [1, G4], F16, kind="ExternalInput").ap()
    bp_d = nc.dram_tensor("bp", [1, V], F16, kind="ExternalInput").ap()
    ident_d = nc.dram_tensor("ident", [P, P], F16, kind="ExternalInput").ap()
    selq_d = nc.dram_tensor("selq", [BL, NBT * P], F16, kind="ExternalInput").ap()
    ones_d = nc.dram_tensor("ones", [1, BL], F16, kind="ExternalInput").ap()
    onec_d = nc.dram_tensor("onec", [P, 1], F16, kind="ExternalInput").ap()

    logits_d = nc.dram_tensor("logits", [BL, V], F32, kind="ExternalOutput").ap()
    hout_d = nc.dram_tensor("hout", [BL, D], F32, kind="ExternalOutput").ap()
    cout_d = nc.dram_tensor("cout", [BL, D], F32, kind="ExternalOutput").ap()

    with tile.TileContext(nc) as tc, ExitStack() as ctx:
        const = ctx.enter_context(tc.tile_pool(name="const", bufs=1))
        wpool = ctx.enter_context(tc.tile_pool(name="wpool", bufs=1))
        encp = ctx.enter_context(tc.tile_pool(name="encp", bufs=4))
        enctp = ctx.enter_context(tc.tile_pool(name="enctp", bufs=3))
        wxp = ctx.enter_context(tc.tile_pool(name="wxp", bufs=7))
        whp = ctx.enter_context(tc.tile_pool(name="whp", bufs=3))
        wpp = ctx.enter_context(tc.tile_pool(name="wpp", bufs=7))
        bpp = ctx.enter_context(tc.tile_pool(name="bpp", bufs=2))
        sb = ctx.enter_context(tc.tile_pool(name="sb", bufs=2))
        persist = ctx.enter_context(tc.tile_pool(name="persist", bufs=1))
        lgp = ctx.enter_context(tc.tile_pool(name="lgp", bufs=2))
        # PSUM budget (8 banks): pt 2 + pk 2 + pbig 4.
        pt = ctx.enter_context(tc.tile_pool(name="pt", bufs=2, space="PSUM"))
        pk = ctx.enter_context(tc.tile_pool(name="pk", bufs=2, space="PSUM"))
        pbig = ctx.enter_context(tc.tile_pool(name="pbig", bufs=1, space="PSUM"))

        # ---- critical-path inputs first (q + first k tile) ----
        hT = persist.tile([P, ND, BL], F16, tag="hT")
        nc.sync.dma_start(hT[:], ht_d.rearrange("(k p) b -> p k b", p=P))
        w1s = wpool.tile([P, ND, A], F16, tag="w1")
        nc.sync.dma_start(w1s[:], w1_d.rearrange("(k p) n -> p k n", p=P))
        b12 = const.tile([1, A], F16, tag="b12")
        nc.sync.dma_start(b12[:], b12_d)
        ones = const.tile([1, BL], F16, tag="ones")
        nc.sync.dma_start(ones[:], ones_d)
        selq = const.tile([BL, NBT * P], F16, tag="selq")
        nc.sync.dma_start(selq[:], selq_d)

        vrow = const.tile([P, A], F16, tag="vrow")
        nc.gpsimd.dma_start(vrow[:], vrow_d)
        bvb = const.tile([P, 1], F32, tag="bvb")
        nc.gpsimd.dma_start(bvb[:], bvb_d)
        onec = const.tile([P, 1], F16, tag="onec")
        nc.gpsimd.dma_start(onec[:], onec_d)
        ident = const.tile([P, P], F16, tag="ident")
        nc.gpsimd.dma_start(ident[:], ident_d)
        bl_sb = const.tile([1, G4], F16, tag="bl")
        nc.gpsimd.dma_start(bl_sb[:], bl_d)
        carry = persist.tile([BL, D], F32, tag="carry")
        nc.gpsimd.dma_start(carry[:], carry_d)
        idxt = persist.tile([BL, 1], dt.int32, tag="idx")
        nc.gpsimd.dma_start(idxt[:], idx_d)

        # embedding gather early (only depends on idxt); copied into lstm_in later
        xg = persist.tile([BL, EMB], F16, tag="xg")
        nc.gpsimd.indirect_dma_start(
            out=xg[:], out_offset=None, in_=emb_d[:],
            in_offset=bass.IndirectOffsetOnAxis(ap=idxt[:, 0:1], axis=0))

        # ---- q = hidden @ w1 + (b1 + b2)   [BL, A] ----
        psq = pk.tile([BL, A], F32, tag="pk")
        for k in range(ND):
            nc.tensor.matmul(psq[:], lhsT=hT[:, k, :], rhs=w1s[:, k, :],
                             start=(k == 0), stop=False)
        nc.tensor.matmul(psq[:], lhsT=ones[:], rhs=b12[:], start=False, stop=True)
        qsb = persist.tile([BL, A], F16, tag="q")
        nc.scalar.copy(qsb[:], psq[:])

        # ---- attention, fully pipelined per bt-tile ----
        score_all = persist.tile([P, NBT], F32, tag="score")
        p_all = persist.tile([P, NBT], F32, tag="p_all")
        Wp = persist.tile([P, NBT * BL], F16, tag="Wp")
        nc.vector.memset(Wp[:], 0.0)
        psctx = pbig.tile([BL, E], F32, tag="pbig")
        psz = pt.tile([BL, 1], F32, tag="pt")
        enc_tiles = []
        w2s = []

        def _ctx_mms(jj):
            for c in range(4):
                nc.tensor.matmul(psctx[:, c * 512:(c + 1) * 512],
                                 lhsT=Wp[:, jj * BL:(jj + 1) * BL],
                                 rhs=enc_tiles[jj][:, c * 512:(c + 1) * 512],
                                 start=(jj == 0), stop=(jj == NBT - 1))
            nc.tensor.matmul(psz[:], lhsT=Wp[:, jj * BL:(jj + 1) * BL], rhs=onec[:],
                             start=(jj == 0), stop=(jj == NBT - 1))

        for j in range(NBT):
            encT = enctp.tile([P, NE, P], F16, tag="encT")
            nc.sync.dma_start(encT[:], encT_d[j].rearrange("p (t c) -> p t c", c=P))
            et = encp.tile([P, E], F16, tag="enc")
            nc.sync.dma_start(et[:], enc_d[j * P:(j + 1) * P, :])
            enc_tiles.append(et)
            if j == 0:
                # w2 after the first enc tiles: 4 chunked tiles so the first
                # k-projection matmul only waits on the first 0.5 MB
                for w2i in range(4):
                    w2c = wpool.tile([P, 4, A], F16, tag=f"w2_{w2i}")
                    nc.sync.dma_start(
                        w2c[:], w2_d.rearrange("(k p) n -> p k n", p=P)[:, 4 * w2i:4 * w2i + 4, :])
                    w2s.append(w2c)
                # gate the gpsimd weight-prefetch stream behind the first
                # critical-path encT tile so it doesn't steal startup bandwidth
                gate = persist.tile([1, 1], F16, tag="gate")
                nc.gpsimd.tensor_copy(gate[:], encT[0:1, 0, 0:1])

            # karg[bt, a] = q[b(bt)] + enc[bt, :] @ w2  (+ b1 + b2 via q)
            psk = pk.tile([P, A], F32, tag="pk")
            nc.tensor.matmul(psk[:], lhsT=selq[:, j * P:(j + 1) * P], rhs=qsb[:],
                             start=True, stop=False)
            for e in range(NE):
                nc.tensor.matmul(psk[:], lhsT=encT[:, e, :],
                                 rhs=w2s[e // 4][:, e % 4, :],
                                 start=False, stop=(e == NE - 1))

            # context for the PREVIOUS tile, emitted before this tile's score
            # chain so the PE does not wait on it through the Wp dependency
            if j > 0:
                _ctx_mms(j - 1)

            tt = sb.tile([P, A], F16, tag="tanh")
            nc.scalar.activation(tt[:], psk[:], AF.Tanh)

            # score[bt] = sum_a tanh(karg) * v ; p = exp(score + bv)
            scratch = sb.tile([P, A], F16, tag="scratch")
            nc.vector.scalar_tensor_tensor(
                out=scratch[:], in0=tt[:], scalar=1.0, in1=vrow[:],
                op0=ALU.mult, op1=ALU.mult,
                accum_out=score_all[:, j:j + 1])
            nc.scalar.activation(p_all[:, j:j + 1], score_all[:, j:j + 1],
                                 AF.Exp, bias=bvb[:])

            # block-sparse attention-weight columns for this tile
            nc.vector.tensor_copy(Wp[0:64, j * BL + 2 * j:j * BL + 2 * j + 1],
                                  p_all[0:64, j:j + 1])
            nc.vector.tensor_copy(Wp[64:128, j * BL + 2 * j + 1:j * BL + 2 * j + 2],
                                  p_all[64:128, j:j + 1])
        _ctx_mms(NBT - 1)

        rz = sb.tile([BL, 1], F32, tag="rz")
        nc.vector.reciprocal(rz[:], psz[:])

        # ---- lstm_in = [context / Z, x] ----
        lstm_in = persist.tile([BL, EI], F16, tag="lstm_in")
        nc.vector.tensor_copy(lstm_in[:, E:EI], xg[:])

        # ---- normalize + transpose lstm_in -> [EI, BL], chunk-pipelined ----
        linT = persist.tile([P, NEI * BL], F16, tag="linT")
        for g in range(5):
            nblk = 4 if g < 4 else 2
            if g < 4:
                nc.vector.tensor_scalar_mul(lstm_in[:, g * 512:(g + 1) * 512],
                                            psctx[:, g * 512:(g + 1) * 512], rz[:])
            ptr_ = pt.tile([P, 512], F16, tag="pt")
            for t_ in range(nblk):
                kk = g * 4 + t_
                nc.tensor.transpose(ptr_[:, t_ * BL:(t_ + 1) * BL],
                                    lstm_in[:, kk * P:(kk + 1) * P],
                                    ident[0:BL, 0:BL])
            nc.scalar.copy(linT[:, g * 4 * BL:(g * 4 + nblk) * BL],
                           ptr_[:, 0:nblk * BL])

        # ---- z = lstm_in @ wx + hidden @ wh + bl   [BL, 4D] ----
        psz2 = pbig.tile([BL, G4], F32, tag="pbig")
        for kk in range(NEI):
            wxt = wxp.tile([P, G4], F16, tag="wx")
            nc.gpsimd.dma_start(wxt[:], wx_d[kk * P:(kk + 1) * P, :])
            for c in range(4):
                nc.tensor.matmul(psz2[:, c * 512:(c + 1) * 512],
                                 lhsT=linT[:, kk * BL:(kk + 1) * BL],
                                 rhs=wxt[:, c * 512:(c + 1) * 512],
                                 start=(kk == 0), stop=False)
        for k in range(ND):
            wht = whp.tile([P, G4], F16, tag="wh")
            nc.gpsimd.dma_start(wht[:], wh_d[k * P:(k + 1) * P, :])
            for c in range(4):
                nc.tensor.matmul(psz2[:, c * 512:(c + 1) * 512],
                                 lhsT=hT[:, k, :], rhs=wht[:, c * 512:(c + 1) * 512],
                                 start=False, stop=False)
        for c in range(4):
            nc.tensor.matmul(psz2[:, c * 512:(c + 1) * 512],
                             lhsT=ones[:], rhs=bl_sb[:, c * 512:(c + 1) * 512],
                             start=False, stop=True)

        # ---- LSTM gates (order i, f, g, o) ----
        sig_i = persist.tile([BL, D], F32, tag="g0")
        sig_f = persist.tile([BL, D], F32, tag="g1")
        tanh_g = persist.tile([BL, D], F32, tag="g2")
        sig_o = persist.tile([BL, D], F32, tag="g3")
        nc.scalar.activation(sig_i[:], psz2[:, 0:D], AF.Sigmoid)
        nc.scalar.activation(sig_f[:], psz2[:, D:2 * D], AF.Sigmoid)
        nc.scalar.activation(tanh_g[:], psz2[:, 2 * D:3 * D], AF.Tanh)
        nc.scalar.activation(sig_o[:], psz2[:, 3 * D:4 * D], AF.Sigmoid)

        cnew = persist.tile([BL, D], F32, tag="cnew")
        nc.vector.tensor_mul(cnew[:], sig_f[:], carry[:])
        nc.vector.tensor_mul(tanh_g[:], sig_i[:], tanh_g[:])
        nc.vector.tensor_add(cnew[:], cnew[:], tanh_g[:])
        nc.sync.dma_start(cout_d, cnew[:])

        tanh_c = persist.tile([BL, D], F32, tag="tanhc")
        nc.scalar.activation(tanh_c[:], cnew[:], AF.Tanh)
        hnew = persist.tile([BL, D], F32, tag="hnew")
        nc.vector.tensor_mul(hnew[:], sig_o[:], tanh_c[:])
        nc.sync.dma_start(hout_d, hnew[:])

        hnew16 = persist.tile([BL, D], F16, tag="h16")
        nc.vector.tensor_copy(hnew16[:], hnew[:])
        hT2 = persist.tile([P, ND * BL], F16, tag="hT2")
        ptr_ = pt.tile([P, 512], F16, tag="pt")
        for k in range(ND):
            nc.tensor.transpose(ptr_[:, k * BL:(k + 1) * BL],
                                hnew16[:, k * P:(k + 1) * P], ident[0:BL, 0:BL])
        nc.scalar.copy(hT2[:], ptr_[:, 0:ND * BL])

        # ---- logits = h_new @ wp + bp   [BL, V] ----
        for gi in range(5):
            lsb = lgp.tile([BL, 2048], F32, tag="lg")
            for cj in range(4):
                ci = gi * 4 + cj
                n0 = ci * 512
                w = min(512, V - n0)
                wpt = wpp.tile([P, ND, 512], F16, tag="wp")
                nc.gpsimd.dma_start(wpt[:, :, 0:w],
                                    wp_d.rearrange("(k p) n -> p k n", p=P)[:, :, n0:n0 + w])
                bpc = bpp.tile([1, 512], F16, tag="bpc")
                nc.gpsimd.dma_start(bpc[:, 0:w], bp_d[0:1, n0:n0 + w])
                pl = pk.tile([BL, 512], F32, tag="pk")
                for k in range(ND):
                    nc.tensor.matmul(pl[:, 0:w], lhsT=hT2[:, k * BL:(k + 1) * BL],
                                     rhs=wpt[:, k, 0:w], start=(k == 0), stop=False)
                nc.tensor.matmul(pl[:, 0:w], lhsT=ones[:], rhs=bpc[:, 0:w],
                                 start=False, stop=True)
                if ci % 2 == 0:
                    nc.scalar.copy(lsb[:, cj * 512:cj * 512 + w], pl[:, 0:w])
                else:
                    nc.vector.tensor_copy(lsb[:, cj * 512:cj * 512 + w], pl[:, 0:w])
            g0_ = gi * 2048
            gw = min(2048, V - g0_)
            nc.sync.dma_start(logits_d[:, g0_:g0_ + gw], lsb[:, 0:gw])

    nc.compile()
    _CACHE["nc"] = nc
    return nc


def _make_inmaps(inputs):
    """Host-side sharding + layout prep. Returns list of per-core input dicts."""
    f16 = np.float16
    f32 = np.float32

    inp = {k: np.asarray(v) for k, v in inputs.items()}
    enc = np.asarray(inp["encoder_output"], dtype=f32)          # [B, T, E]
    hidden = np.asarray(inp["hidden"], dtype=f32)               # [B, D]
    carry = np.asarray(inp["carry"], dtype=f32)                 # [B, D]
    idx = np.asarray(inp["input_c"]).astype(np.int32).reshape(B, 1)
    emb = np.asarray(inp["emb"], dtype=f32).astype(f16)         # [V, EMB]
    w1 = np.asarray(inp["w1"], dtype=f32).astype(f16)
    w2 = np.asarray(inp["w2"], dtype=f32).astype(f16)
    wx = np.asarray(inp["wx"], dtype=f32).astype(f16)
    wh = np.asarray(inp["wh"], dtype=f32).astype(f16)
    wp = np.asarray(inp["wp"], dtype=f32).astype(f16)
    v = np.asarray(inp["v"], dtype=f32)                         # [A, 1]
    b1 = np.asarray(inp["b1"], dtype=f32)
    b2 = np.asarray(inp["b2"], dtype=f32)
    bv = np.asarray(inp["bv"], dtype=f32)
    bl = np.asarray(inp["bl"], dtype=f32)
    bp = np.asarray(inp["bp"], dtype=f32)

    enc16 = enc.astype(f16)
    hidden_t16 = np.ascontiguousarray(hidden.T).astype(f16)     # [D, B]

    ident = np.eye(P, dtype=f16)
    # selq[b, j*P + c] = 1 iff b == 2j + (c >= 64)
    selq = np.zeros((BL, NBT * P), dtype=f16)
    for j in range(NBT):
        selq[2 * j, j * P:j * P + 64] = 1.0
        selq[2 * j + 1, j * P + 64:(j + 1) * P] = 1.0
    ones = np.ones((1, BL), dtype=f16)
    onec = np.ones((P, 1), dtype=f16)
    vrow = np.ascontiguousarray(np.broadcast_to(v.reshape(1, A), (P, A))).astype(f16)
    b12 = (b1 + b2).reshape(1, A).astype(f16)
    bvb = np.full((P, 1), float(bv.reshape(-1)[0]), dtype=f32)
    bl_r = bl.reshape(1, G4).astype(f16)
    bp_r = bp.reshape(1, V).astype(f16)

    in_maps = []
    for i in range(NCORES):
        b0, b1_ = i * BL, (i + 1) * BL
        enc_i = np.ascontiguousarray(enc16[b0:b1_].reshape(BT, E))
        # encT[j, p, t, c] = enc_i[j*128+c, t*128+p]  -> [NBT, P, E]
        encT_i = np.ascontiguousarray(
            enc_i.reshape(NBT, P, NE, P).transpose(0, 3, 2, 1).reshape(NBT, P, E))
        in_maps.append({
            "enc": enc_i,
            "encT": encT_i,
            "ht": np.ascontiguousarray(hidden_t16[:, b0:b1_]),
            "carry": np.ascontiguousarray(carry[b0:b1_]),
            "idx": np.ascontiguousarray(idx[b0:b1_]),
            "emb": emb,
            "w1": w1, "w2": w2, "wx": wx, "wh": wh, "wp": wp,
            "vrow": vrow, "b12": b12, "bvb": bvb,
            "bl": bl_r, "bp": bp_r,
            "ident": ident, "selq": selq, "ones": ones, "onec": onec,
        })
    return in_maps


def kernel(**inputs):
    global LAST_RESULTS
    _ensure_concourse()
    from concourse import bass_utils

    nc = _build()
    in_maps = _make_inmaps(inputs)
    res = bass_utils.run_bass_kernel_spmd(nc, in_maps, core_ids=list(range(NCORES)))
    LAST_RESULTS = res

    logits = np.concatenate([r["logits"] for r in res.results], axis=0)
    h_new = np.concatenate([r["hout"] for r in res.results], axis=0)
    c_new = np.concatenate([r["cout"] for r in res.results], axis=0)
    return logits, h_new, c_new


# revision 15
# speedup vs baseline: 1.0397x; 1.0397x over previous
"""Trainium2 Bass kernel for nn_Decoder_72438918414350.

Single decoder step: embedding lookup + Bahdanau additive attention over
encoder outputs + LSTM cell + vocab projection.

Sharding: data-parallel over batch across 8 NeuronCores (32 rows/core).
All weights are replicated per core; matmul operands in fp16 with fp32
accumulation in PSUM; gate math and outputs in fp32. The host supplies the
encoder output both in natural layout (for the attention-weighted context
sum) and pre-transposed per bt-tile (for the E-contraction of the score
projection), so no on-chip transposes of the large tensor are needed.

Self-contained: hardcodes all shapes; only imports the installed concourse
runtime.
"""

import os
import sys
from contextlib import ExitStack

import numpy as np


def _ensure_concourse():
    try:
        import concourse.bass  # noqa: F401
    except ImportError:
        for p in ("/opt/trn_rl_repo", "/root/.axon_site/_ro/trn_rl_repo"):
            if os.path.isdir(p) and p not in sys.path:
                sys.path.insert(0, p)
        import concourse.bass  # noqa: F401


P = 128
B, T, E, D, A, V, EMB = 256, 64, 2048, 512, 512, 10000, 256
NCORES = 8
BL = B // NCORES            # 32 batch rows per core
BT = BL * T                 # 2048 (batch*time rows per core)
NBT = BT // P               # 16 bt tiles
NE = E // P                 # 16 e tiles
EI = E + EMB                # 2304
NEI = EI // P               # 18
G4 = 4 * D                  # 2048
ND = D // P                 # 4
NV512 = (V + 511) // 512    # 20 vocab chunks of <=512

_CACHE = {}
LAST_RESULTS = None


def _build():
    """Build + compile the Bass module (cached)."""
    if "nc" in _CACHE:
        return _CACHE["nc"]

    import concourse.bass as bass
    import concourse.mybir as mybir
    import concourse.tile as tile
    from concourse import bacc

    dt = mybir.dt
    AF = mybir.ActivationFunctionType
    ALU = mybir.AluOpType
    F16 = dt.float16
    F32 = dt.float32

    nc = bacc.Bacc("TRN2", target_bir_lowering=False, debug=False)

    # ---- DRAM tensors (per-core shard shapes) ----
    enc_d = nc.dram_tensor("enc", [BT, E], F16, kind="ExternalInput").ap()
    # encT[j, p, t*128+c] = enc[j*128+c, t*128+p]: per-tile transposed, partition-major
    encT_d = nc.dram_tensor("encT", [NBT, P, E], F16, kind="ExternalInput").ap()
    ht_d = nc.dram_tensor("ht", [D, BL], F16, kind="ExternalInput").ap()
    carry_d = nc.dram_tensor("carry", [BL, D], F32, kind="ExternalInput").ap()
    idx_d = nc.dram_tensor("idx", [BL, 1], dt.int32, kind="ExternalInput").ap()
    emb_d = nc.dram_tensor("emb", [V, EMB], F16, kind="ExternalInput").ap()
    w1_d = nc.dram_tensor("w1", [D, A], F16, kind="ExternalInput").ap()
    w2_d = nc.dram_tensor("w2", [E, A], F16, kind="ExternalInput").ap()
    wx_d = nc.dram_tensor("wx", [EI, G4], F16, kind="ExternalInput").ap()
    wh_d = nc.dram_tensor("wh", [D, G4], F16, kind="ExternalInput").ap()
    wp_d = nc.dram_tensor("wp", [D, V], F16, kind="ExternalInput").ap()
    vrow_d = nc.dram_tensor("vrow", [P, A], F16, kind="ExternalInput").ap()
    b12_d = nc.dram_tensor("b12", [1, A], F16, kind="ExternalInput").ap()
    bvb_d = nc.dram_tensor("bvb", [P, 1], F32, kind="ExternalInput").ap()
    bl_d = nc.dram_tensor("bl", [1, G4], F16, kind="ExternalInput").ap()
    bp_d = nc.dram_tensor("bp", [1, V], F16, kind="ExternalInput").ap()
    ident_d = nc.dram_tensor("ident", [P, P], F16, kind="ExternalInput").ap()
    selq_d = nc.dram_tensor("selq", [BL, NBT * P], F16, kind="ExternalInput").ap()
    ones_d = nc.dram_tensor("ones", [1, BL], F16, kind="ExternalInput").ap()
    onec_d = nc.dram_tensor("onec", [P, 1], F16, kind="ExternalInput").ap()

    logits_d = nc.dram_tensor("logits", [BL, V], F32, kind="ExternalOutput").ap()
    hout_d = nc.dram_tensor("hout", [BL, D], F32, kind="ExternalOutput").ap()
    cout_d = nc.dram_tensor("cout", [BL, D], F32, kind="ExternalOutput").ap()

    with tile.TileContext(nc) as tc, ExitStack() as ctx:
        const = ctx.enter_context(tc.tile_pool(name="const", bufs=1))
        wpool = ctx.enter_context(tc.tile_pool(name="wpool", bufs=1))
        encp = ctx.enter_context(tc.tile_pool(name="encp", bufs=5))
        enctp = ctx.enter_context(tc.tile_pool(name="enctp", bufs=3))
        wxp = ctx.enter_context(tc.tile_pool(name="wxp", bufs=7))
        whp = ctx.enter_context(tc.tile_pool(name="whp", bufs=3))
        wpp = ctx.enter_context(tc.tile_pool(name="wpp", bufs=4))
        bpp = ctx.enter_context(tc.tile_pool(name="bpp", bufs=2))
        sb = ctx.enter_context(tc.tile_pool(name="sb", bufs=2))
        persist = ctx.enter_context(tc.tile_pool(name="persist", bufs=1))
        lgp = ctx.enter_context(tc.tile_pool(name="lgp", bufs=2))
        # PSUM budget (8 banks): pt 2 + pk 2 + pbig 4.
        pt = ctx.enter_context(tc.tile_pool(name="pt", bufs=2, space="PSUM"))
        pk = ctx.enter_context(tc.tile_pool(name="pk", bufs=2, space="PSUM"))
        pbig = ctx.enter_context(tc.tile_pool(name="pbig", bufs=1, space="PSUM"))

        # ---- critical-path inputs first (q + first k tile) ----
        hT = persist.tile([P, ND, BL], F16, tag="hT")
        nc.sync.dma_start(hT[:], ht_d.rearrange("(k p) b -> p k b", p=P))
        w1s = wpool.tile([P, ND, A], F16, tag="w1")
        nc.sync.dma_start(w1s[:], w1_d.rearrange("(k p) n -> p k n", p=P))
        b12 = const.tile([1, A], F16, tag="b12")
        nc.gpsimd.dma_start(b12[:], b12_d)
        ones = const.tile([1, BL], F16, tag="ones")
        nc.gpsimd.dma_start(ones[:], ones_d)
        selq = const.tile([BL, NBT * P], F16, tag="selq")
        nc.gpsimd.dma_start(selq[:], selq_d)

        vrow = const.tile([P, A], F16, tag="vrow")
        nc.gpsimd.dma_start(vrow[:], vrow_d)
        bvb = const.tile([P, 1], F32, tag="bvb")
        nc.gpsimd.dma_start(bvb[:], bvb_d)
        onec = const.tile([P, 1], F16, tag="onec")
        nc.gpsimd.dma_start(onec[:], onec_d)
        ident = const.tile([P, P], F16, tag="ident")
        nc.gpsimd.dma_start(ident[:], ident_d)
        bl_sb = const.tile([1, G4], F16, tag="bl")
        nc.gpsimd.dma_start(bl_sb[:], bl_d)
        carry = persist.tile([BL, D], F32, tag="carry")
        nc.gpsimd.dma_start(carry[:], carry_d)
        idxt = persist.tile([BL, 1], dt.int32, tag="idx")
        nc.gpsimd.dma_start(idxt[:], idx_d)

        # embedding gather early (only depends on idxt); copied into lstm_in later
        xg = persist.tile([BL, EMB], F16, tag="xg")
        nc.gpsimd.indirect_dma_start(
            out=xg[:], out_offset=None, in_=emb_d[:],
            in_offset=bass.IndirectOffsetOnAxis(ap=idxt[:, 0:1], axis=0))

        # ---- q = hidden @ w1 + (b1 + b2)   [BL, A] ----
        psq = pk.tile([BL, A], F32, tag="pk")
        for k in range(ND):
            nc.tensor.matmul(psq[:], lhsT=hT[:, k, :], rhs=w1s[:, k, :],
                             start=(k == 0), stop=False)
        nc.tensor.matmul(psq[:], lhsT=ones[:], rhs=b12[:], start=False, stop=True)
        qsb = persist.tile([BL, A], F16, tag="q")
        nc.scalar.copy(qsb[:], psq[:])

        # ---- attention, fully pipelined per bt-tile ----
        score_all = persist.tile([P, NBT], F32, tag="score")
        p_all = persist.tile([P, NBT], F32, tag="p_all")
        Wp = persist.tile([P, NBT * BL], F16, tag="Wp")
        nc.vector.memset(Wp[:], 0.0)
        psctx = pbig.tile([BL, E], F32, tag="pbig")
        psz = pt.tile([BL, 1], F32, tag="pt")
        enc_tiles = []
        w2s = []

        def _ctx_mms(jj):
            for c in range(4):
                nc.tensor.matmul(psctx[:, c * 512:(c + 1) * 512],
                                 lhsT=Wp[:, jj * BL:(jj + 1) * BL],
                                 rhs=enc_tiles[jj][:, c * 512:(c + 1) * 512],
                                 start=(jj == 0), stop=(jj == NBT - 1))
            nc.tensor.matmul(psz[:], lhsT=Wp[:, jj * BL:(jj + 1) * BL], rhs=onec[:],
                             start=(jj == 0), stop=(jj == NBT - 1))

        for j in range(NBT):
            encT = enctp.tile([P, NE, P], F16, tag="encT")
            nc.sync.dma_start(encT[:], encT_d[j].rearrange("p (t c) -> p t c", c=P))
            et = encp.tile([P, E], F16, tag="enc")
            nc.sync.dma_start(et[:], enc_d[j * P:(j + 1) * P, :])
            enc_tiles.append(et)
            if j == 0:
                # w2 after the first enc tiles: 4 chunked tiles so the first
                # k-projection matmul only waits on the first 0.5 MB
                for w2i in range(4):
                    w2c = wpool.tile([P, 4, A], F16, tag=f"w2_{w2i}")
                    nc.sync.dma_start(
                        w2c[:], w2_d.rearrange("(k p) n -> p k n", p=P)[:, 4 * w2i:4 * w2i + 4, :])
                    w2s.append(w2c)
                # gate the gpsimd weight-prefetch stream behind the first
                # critical-path encT tile so it doesn't steal startup bandwidth
                gate = persist.tile([1, 1], F16, tag="gate")
                nc.gpsimd.tensor_copy(gate[:], encT[0:1, 0, 0:1])

            # karg[bt, a] = q[b(bt)] + enc[bt, :] @ w2  (+ b1 + b2 via q)
            psk = pk.tile([P, A], F32, tag="pk")
            for e in range(NE):
                nc.tensor.matmul(psk[:], lhsT=encT[:, e, :],
                                 rhs=w2s[e // 4][:, e % 4, :],
                                 start=(e == 0), stop=False)
            nc.tensor.matmul(psk[:], lhsT=selq[:, j * P:(j + 1) * P], rhs=qsb[:],
                             start=False, stop=True)

            # context for an earlier tile, emitted before this tile's score
            # chain so the PE does not wait on it through the Wp dependency
            if j > 1:
                _ctx_mms(j - 2)

            tt = sb.tile([P, A], F16, tag="tanh")
            nc.scalar.activation(tt[:], psk[:], AF.Tanh)

            # score[bt] = sum_a tanh(karg) * v ; p = exp(score + bv)
            scratch = sb.tile([P, A], F16, tag="scratch")
            nc.vector.scalar_tensor_tensor(
                out=scratch[:], in0=tt[:], scalar=1.0, in1=vrow[:],
                op0=ALU.mult, op1=ALU.mult,
                accum_out=score_all[:, j:j + 1])
            nc.scalar.activation(p_all[:, j:j + 1], score_all[:, j:j + 1],
                                 AF.Exp, bias=bvb[:])

            # block-sparse attention-weight columns for this tile
            nc.vector.tensor_copy(Wp[0:64, j * BL + 2 * j:j * BL + 2 * j + 1],
                                  p_all[0:64, j:j + 1])
            nc.vector.tensor_copy(Wp[64:128, j * BL + 2 * j + 1:j * BL + 2 * j + 2],
                                  p_all[64:128, j:j + 1])
        _ctx_mms(NBT - 2)
        _ctx_mms(NBT - 1)

        rz = sb.tile([BL, 1], F32, tag="rz")
        nc.vector.reciprocal(rz[:], psz[:])

        # ---- lstm_in = [context / Z, x] ----
        lstm_in = persist.tile([BL, EI], F16, tag="lstm_in")
        nc.vector.tensor_copy(lstm_in[:, E:EI], xg[:])

        # ---- normalize + transpose lstm_in -> [EI, BL], chunk-pipelined ----
        linT = persist.tile([P, NEI * BL], F16, tag="linT")
        for g in range(5):
            nblk = 4 if g < 4 else 2
            if g < 4:
                nc.vector.tensor_scalar_mul(lstm_in[:, g * 512:(g + 1) * 512],
                                            psctx[:, g * 512:(g + 1) * 512], rz[:])
            ptr_ = pt.tile([P, 512], F16, tag="pt")
            for t_ in range(nblk):
                kk = g * 4 + t_
                nc.tensor.transpose(ptr_[:, t_ * BL:(t_ + 1) * BL],
                                    lstm_in[:, kk * P:(kk + 1) * P],
                                    ident[0:BL, 0:BL])
            nc.scalar.copy(linT[:, g * 4 * BL:(g * 4 + nblk) * BL],
                           ptr_[:, 0:nblk * BL])

        # ---- z = lstm_in @ wx + hidden @ wh + bl   [BL, 4D] ----
        psz2 = pbig.tile([BL, G4], F32, tag="pbig")
        for kk in range(NEI):
            wxt = wxp.tile([P, G4], F16, tag="wx")
            nc.gpsimd.dma_start(wxt[:], wx_d[kk * P:(kk + 1) * P, :])
            for c in range(4):
                nc.tensor.matmul(psz2[:, c * 512:(c + 1) * 512],
                                 lhsT=linT[:, kk * BL:(kk + 1) * BL],
                                 rhs=wxt[:, c * 512:(c + 1) * 512],
                                 start=(kk == 0), stop=False)
        for k in range(ND):
            wht = whp.tile([P, G4], F16, tag="wh")
            nc.gpsimd.dma_start(wht[:], wh_d[k * P:(k + 1) * P, :])
            for c in range(4):
                nc.tensor.matmul(psz2[:, c * 512:(c + 1) * 512],
                                 lhsT=hT[:, k, :], rhs=wht[:, c * 512:(c + 1) * 512],
                                 start=False, stop=False)
        for c in range(4):
            nc.tensor.matmul(psz2[:, c * 512:(c + 1) * 512],
                             lhsT=ones[:], rhs=bl_sb[:, c * 512:(c + 1) * 512],
                             start=False, stop=True)

        # ---- LSTM gates (order i, f, g, o) ----
        sig_i = persist.tile([BL, D], F32, tag="g0")
        sig_f = persist.tile([BL, D], F32, tag="g1")
        tanh_g = persist.tile([BL, D], F32, tag="g2")
        sig_o = persist.tile([BL, D], F32, tag="g3")
        nc.scalar.activation(sig_i[:], psz2[:, 0:D], AF.Sigmoid)
        nc.scalar.activation(sig_f[:], psz2[:, D:2 * D], AF.Sigmoid)
        nc.scalar.activation(tanh_g[:], psz2[:, 2 * D:3 * D], AF.Tanh)
        nc.scalar.activation(sig_o[:], psz2[:, 3 * D:4 * D], AF.Sigmoid)

        cnew = persist.tile([BL, D], F32, tag="cnew")
        nc.vector.tensor_mul(cnew[:], sig_f[:], carry[:])
        nc.vector.tensor_mul(tanh_g[:], sig_i[:], tanh_g[:])
        nc.vector.tensor_add(cnew[:], cnew[:], tanh_g[:])
        nc.sync.dma_start(cout_d, cnew[:])

        tanh_c = persist.tile([BL, D], F32, tag="tanhc")
        nc.scalar.activation(tanh_c[:], cnew[:], AF.Tanh)
        hnew = persist.tile([BL, D], F32, tag="hnew")
        nc.vector.tensor_mul(hnew[:], sig_o[:], tanh_c[:])
        nc.sync.dma_start(hout_d, hnew[:])

        hnew16 = persist.tile([BL, D], F16, tag="h16")
        nc.vector.tensor_copy(hnew16[:], hnew[:])
        hT2 = persist.tile([P, ND * BL], F16, tag="hT2")
        ptr_ = pt.tile([P, 512], F16, tag="pt")
        for k in range(ND):
            nc.tensor.transpose(ptr_[:, k * BL:(k + 1) * BL],
                                hnew16[:, k * P:(k + 1) * P], ident[0:BL, 0:BL])
        nc.scalar.copy(hT2[:], ptr_[:, 0:ND * BL])

        # ---- logits = h_new @ wp + bp   [BL, V] ----
        wp_r = wp_d.rearrange("(k p) n -> p k n", p=P)
        for gi in range(5):
            lsb = lgp.tile([BL, 2048], F32, tag="lg")
            for half in range(2):
                nh0 = gi * 2048 + half * 1024
                hw_ = min(1024, V - nh0)
                wpt = wpp.tile([P, ND, 1024], F16, tag="wp")
                nc.gpsimd.dma_start(wpt[:, :, 0:hw_], wp_r[:, :, nh0:nh0 + hw_])
                bpc = bpp.tile([1, 1024], F16, tag="bpc")
                nc.gpsimd.dma_start(bpc[:, 0:hw_], bp_d[0:1, nh0:nh0 + hw_])
                for sub in range(2):
                    n0 = nh0 + sub * 512
                    w = min(512, V - n0)
                    if w <= 0:
                        continue
                    ci = n0 // 512
                    pl = pk.tile([BL, 512], F32, tag="pk")
                    for k in range(ND):
                        nc.tensor.matmul(pl[:, 0:w], lhsT=hT2[:, k * BL:(k + 1) * BL],
                                         rhs=wpt[:, k, sub * 512:sub * 512 + w],
                                         start=(k == 0), stop=False)
                    nc.tensor.matmul(pl[:, 0:w], lhsT=ones[:],
                                     rhs=bpc[:, sub * 512:sub * 512 + w],
                                     start=False, stop=True)
                    lo = n0 - gi * 2048
                    if ci % 2 == 0:
                        nc.scalar.copy(lsb[:, lo:lo + w], pl[:, 0:w])
                    else:
                        nc.vector.tensor_copy(lsb[:, lo:lo + w], pl[:, 0:w])
            g0_ = gi * 2048
            gw = min(2048, V - g0_)
            nc.sync.dma_start(logits_d[:, g0_:g0_ + gw], lsb[:, 0:gw])

    nc.compile()
    _CACHE["nc"] = nc
    return nc


def _make_inmaps(inputs):
    """Host-side sharding + layout prep. Returns list of per-core input dicts."""
    f16 = np.float16
    f32 = np.float32

    inp = {k: np.asarray(v) for k, v in inputs.items()}
    enc = np.asarray(inp["encoder_output"], dtype=f32)          # [B, T, E]
    hidden = np.asarray(inp["hidden"], dtype=f32)               # [B, D]
    carry = np.asarray(inp["carry"], dtype=f32)                 # [B, D]
    idx = np.asarray(inp["input_c"]).astype(np.int32).reshape(B, 1)
    emb = np.asarray(inp["emb"], dtype=f32).astype(f16)         # [V, EMB]
    w1 = np.asarray(inp["w1"], dtype=f32).astype(f16)
    w2 = np.asarray(inp["w2"], dtype=f32).astype(f16)
    wx = np.asarray(inp["wx"], dtype=f32).astype(f16)
    wh = np.asarray(inp["wh"], dtype=f32).astype(f16)
    wp = np.asarray(inp["wp"], dtype=f32).astype(f16)
    v = np.asarray(inp["v"], dtype=f32)                         # [A, 1]
    b1 = np.asarray(inp["b1"], dtype=f32)
    b2 = np.asarray(inp["b2"], dtype=f32)
    bv = np.asarray(inp["bv"], dtype=f32)
    bl = np.asarray(inp["bl"], dtype=f32)
    bp = np.asarray(inp["bp"], dtype=f32)

    enc16 = enc.astype(f16)
    hidden_t16 = np.ascontiguousarray(hidden.T).astype(f16)     # [D, B]

    ident = np.eye(P, dtype=f16)
    # selq[b, j*P + c] = 1 iff b == 2j + (c >= 64)
    selq = np.zeros((BL, NBT * P), dtype=f16)
    for j in range(NBT):
        selq[2 * j, j * P:j * P + 64] = 1.0
        selq[2 * j + 1, j * P + 64:(j + 1) * P] = 1.0
    ones = np.ones((1, BL), dtype=f16)
    onec = np.ones((P, 1), dtype=f16)
    vrow = np.ascontiguousarray(np.broadcast_to(v.reshape(1, A), (P, A))).astype(f16)
    b12 = (b1 + b2).reshape(1, A).astype(f16)
    bvb = np.full((P, 1), float(bv.reshape(-1)[0]), dtype=f32)
    bl_r = bl.reshape(1, G4).astype(f16)
    bp_r = bp.reshape(1, V).astype(f16)

    in_maps = []
    for i in range(NCORES):
        b0, b1_ = i * BL, (i + 1) * BL
        enc_i = np.ascontiguousarray(enc16[b0:b1_].reshape(BT, E))
        # encT[j, p, t, c] = enc_i[j*128+c, t*128+p]  -> [NBT, P, E]
        encT_i = np.ascontiguousarray(
            enc_i.reshape(NBT, P, NE, P).transpose(0, 3, 2, 1).reshape(NBT, P, E))
        in_maps.append({
            "enc": enc_i,
            "encT": encT_i,
            "ht": np.ascontiguousarray(hidden_t16[:, b0:b1_]),
            "carry": np.ascontiguousarray(carry[b0:b1_]),
            "idx": np.ascontiguousarray(idx[b0:b1_]),
            "emb": emb,
            "w1": w1, "w2": w2, "wx": wx, "wh": wh, "wp": wp,
            "vrow": vrow, "b12": b12, "bvb": bvb,
            "bl": bl_r, "bp": bp_r,
            "ident": ident, "selq": selq, "ones": ones, "onec": onec,
        })
    return in_maps


def kernel(**inputs):
    global LAST_RESULTS
    _ensure_concourse()
    from concourse import bass_utils

    nc = _build()
    in_maps = _make_inmaps(inputs)
    res = bass_utils.run_bass_kernel_spmd(nc, in_maps, core_ids=list(range(NCORES)))
    LAST_RESULTS = res

    logits = np.concatenate([r["logits"] for r in res.results], axis=0)
    h_new = np.concatenate([r["hout"] for r in res.results], axis=0)
    c_new = np.concatenate([r["cout"] for r in res.results], axis=0)
    return logits, h_new, c_new
